# revision 47
# baseline (speedup 1.0000x reference)
"""Trainium2 Bass kernel for nn_EncoderLayer (D=1024, H=16, S=2048, FF=4096), 8-core SPMD.

Strategy: head-parallel attention (2 heads/core), fp16 AllToAll per head to
switch to sequence-parallel (256 positions/core) for the output projection,
norms and FFN. No all-reduce needed anywhere.

Precision: fp16 matmul datapath (1 PE cycle/row, fp32 PSUM accumulation,
fp32 LayerNorm/softmax arithmetic). Only exp(scores) and the V/AV path use
bf16 (e values reach ~e^20, past fp16 range). rel_err vs fp32 ref ~3e-3.

Schedule: per-head A2A overlapped with the other head's compute; the
diagonal mask is matmul-accumulated (-1e5*I) into the score PSUM so no
gpsimd select sits in the exp->AV chain; the t-loop is software-pipelined
(scores(t+1) issues before AV(t)); LayerNorm uses bn_stats + fused
tensor_scalar ops; bodies are software-pipelined across reps: phase A of
body r+1 is emitted inside body r's A2A window so the PE never drains.
"""
import math
import os

import numpy as np
import ml_dtypes

import concourse.bass as bass
import concourse.mybir as mybir
import concourse.tile as tile
from concourse import bacc
from concourse.bass_utils import run_bass_kernel_spmd
from concourse.masks import make_identity

F32 = mybir.dt.float32
F32R = mybir.dt.float32r  # TF32-like PE mode: 1 cycle/row for >=256-row moving ops
F16 = mybir.dt.float16    # same speed/bytes as bf16, 3 more mantissa bits
F16NP = np.float16
BF16 = mybir.dt.bfloat16
AF = mybir.ActivationFunctionType
BF16NP = ml_dtypes.bfloat16

D = 1024
H = 16
HD = 64
S = 2048
FF = 4096
EPS = 1e-3
NCORES = 8
SL = S // NCORES          # 256 sequence positions per core after A2A
HPC = H // NCORES         # 2 heads per core
KT = D // 128             # 8 k-tiles over the model dim
TT = S // 128             # 16 t-tiles over sequence
SCH = 512                 # PSUM-bank-limited matmul output chunk
NSCH = S // SCH           # 4 s-chunks
FFT = FF // 128           # 32 hidden tiles
ISCALE = 1.0 / math.sqrt(HD)


def _ln_norm(nc, pools, x_sb, tag):
    """In-place x <- (x - mu) / (std_ddof1 + eps) over the free axis (1024).

    bn_stats gives mean+variance in one pass (two 512 groups + aggregate);
    the ddof=1 correction folds into the sqrt's scale.
    """
    stats = pools.tile([128, 2, 6], F32, tag="ln_st", name=f"ln_st_{tag}")
    for g in range(2):
        nc.vector.bn_stats(stats[:, g, :], x_sb[:, g * 512:(g + 1) * 512])
    mv = pools.tile([128, 2], F32, tag="ln_mv", name=f"ln_mv_{tag}")
    nc.vector.bn_aggr(mv[:], stats[:])
    sig = pools.tile([128, 1], F32, tag="ln_sig", name=f"ln_sig_{tag}")
    # sigma_ddof1 = sqrt(var * D/(D-1)); then += eps; then reciprocal
    nc.scalar.activation(sig[:], mv[:, 1:2], AF.Sqrt, scale=float(D) / (D - 1))
    nc.vector.tensor_scalar_add(sig[:], sig[:], EPS)
    rec = pools.tile([128, 1], F32, tag="ln_rec", name=f"ln_rec_{tag}")
    nc.vector.reciprocal(rec[:], sig[:])
    # in-place: x_sb <- (x - mu) * rec
    nc.vector.tensor_scalar(out=x_sb[:], in0=x_sb[:], scalar1=mv[:, 0:1],
                            scalar2=rec[:], op0=mybir.AluOpType.subtract,
                            op1=mybir.AluOpType.mult)


def _ln(nc, pools, x_sb, z_sb, a2_sb, b2n_sb, tag):
    """Full LayerNorm: z = (x - mu)/(std_ddof1 + eps) * a2 + b2."""
    _ln_norm(nc, pools, x_sb, tag)
    nc.vector.scalar_tensor_tensor(out=z_sb[:], in0=x_sb[:], scalar=1.0,
                                   in1=a2_sb[:], op0=mybir.AluOpType.mult,
                                   op1=mybir.AluOpType.mult)
    nc.vector.tensor_add(out=z_sb[:], in0=z_sb[:], in1=b2n_sb[:])


class _Ctx:
    """Program-level pools + parameters shared by all pipelined bodies."""


PHASE_MARKS = []  # (first instruction id, phase label) — for sim profiling


def build(reps: int = 1):
    nc = bacc.Bacc("TRN2", target_bir_lowering=False, debug=False, num_devices=NCORES)
    c = _Ctx()
    c.nc = nc

    # ---- DRAM parameters (per-core shards prepared on host, bf16 datapath) ----
    c.Qt = nc.declare_dram_parameter("Qt", [KT, 128, S], F16, isOutput=False)
    c.Kt = nc.declare_dram_parameter("Kt", [KT, 128, S], F16, isOutput=False)
    c.Vt = nc.declare_dram_parameter("Vt", [KT, 128, S], BF16, isOutput=False)
    c.wqT = nc.declare_dram_parameter("wqT", [128, KT, 128], F16, isOutput=False)
    c.wkT = nc.declare_dram_parameter("wkT", [128, KT, 128], F16, isOutput=False)
    c.wvT = nc.declare_dram_parameter("wvT", [128, KT, 128], BF16, isOutput=False)
    c.Wot = nc.declare_dram_parameter("Wot", [128, KT, D], F16, isOutput=False)
    # W1g[g] : [128(ki), 4(m-sub), 8(kt), 128(m)]  contiguous 1MB bf16 blocks
    c.W1g = nc.declare_dram_parameter("W1g", [8, 128, 4, KT, 128], F16, isOutput=False)
    # W2g[g] : [128(ki over f), 4(kt-sub over f), 1024(d)] contiguous 1MB blocks
    c.W2g = nc.declare_dram_parameter("W2g", [8, 128, 4, D], F16, isOutput=False)
    c.b1t = nc.declare_dram_parameter("b1t", [128, FFT], F32, isOutput=False)
    b2v = nc.declare_dram_parameter("b2v", [1, D], F32, isOutput=False)
    a2v = nc.declare_dram_parameter("a2v", [1, D], F32, isOutput=False)
    b2nv = nc.declare_dram_parameter("b2nv", [1, D], F32, isOutput=False)
    c.VsT = nc.declare_dram_parameter("VsT", [2, 128, D], F32, isOutput=False)
    # all reps write the same output buffer: extra reps add zero host
    # transfer, so reps-diff timing isolates device-side work
    out0 = nc.declare_dram_parameter("out0", [2, 128, D], F32, isOutput=True)
    c.outs = [out0] * reps

    with tile.TileContext(nc) as tc:
        c.tc = tc
        import contextlib
        with contextlib.ExitStack() as st:
            ent = st.enter_context
            c.singles = ent(tc.tile_pool(name="singles", bufs=1))
            c.persist = ent(tc.tile_pool(name="persist", bufs=1))
            c.projw = ent(tc.tile_pool(name="projw", bufs=1))
            c.projin = ent(tc.tile_pool(name="projin", bufs=8))
            c.esb = ent(tc.tile_pool(name="esb", bufs=3))
            c.avsb = ent(tc.tile_pool(name="avsb", bufs=2))
            c.lnsb = ent(tc.tile_pool(name="lnsb", bufs=1))
            c.w1s = ent(tc.tile_pool(name="w1s", bufs=2))
            c.w2s = ent(tc.tile_pool(name="w2s", bufs=2))
            c.dram = ent(tc.tile_pool(name="dram", bufs=2, space="DRAM"))

            ident = c.singles.tile([128, 128], BF16)
            make_identity(nc, ident[:])
            identf = c.singles.tile([128, 128], F32)
            make_identity(nc, identf[:])
            # -1e5 * I — matmul-accumulated into the score PSUM to apply the
            # diagonal attention mask without a gpsimd select in the chain
            mdiag = c.singles.tile([128, 128], BF16)
            nc.gpsimd.memset(mdiag[:], 0.0)
            nc.gpsimd.affine_select(
                out=mdiag[:], in_=mdiag[:],
                compare_op=mybir.AluOpType.not_equal,
                fill=-100000.0, base=0,
                pattern=[[-1, 128]], channel_multiplier=1,
            )
            c.mdiag = mdiag
            c.ident, c.identf = ident, identf
            c.a2_sb = c.singles.tile([128, D], F32)
            c.b2n_sb = c.singles.tile([128, D], F32)
            c.b2f_sb = c.singles.tile([128, D], F32)
            c.b1_sb = c.singles.tile([128, FFT], F32)
            vrow = c.singles.tile([1, D], F32)
            nc.sync.dma_start(vrow[:], a2v[:])
            nc.gpsimd.partition_broadcast(c.a2_sb[:], vrow[:])
            nc.sync.dma_start(vrow[:], b2nv[:])
            nc.gpsimd.partition_broadcast(c.b2n_sb[:], vrow[:])
            nc.sync.dma_start(vrow[:], b2v[:])
            nc.gpsimd.partition_broadcast(c.b2f_sb[:], vrow[:])
            nc.sync.dma_start(c.b1_sb[:], c.b1t[:])

            # Pipelined emission: A(0) B(0) | A(1) C(0) D(0) B(1) | A(2) C(1) ...
            # Body r+1's projections run inside body r's A2A window.
            def mark(label):
                PHASE_MARKS.append((int(nc.next_id()), label))

            PHASE_MARKS.clear()
            state = {}
            mark("A0")
            emit_A(c, state, 0)
            mark("B0")
            emit_B(c, state, 0)
            for r in range(1, reps):
                mark(f"A{r}")
                emit_A(c, state, r)
                mark(f"C{r-1}")
                emit_C(c, state, r - 1)
                mark(f"D{r-1}")
                emit_D(c, state, r - 1)
                mark(f"B{r}")
                emit_B(c, state, r)
            mark(f"C{reps-1}")
            emit_C(c, state, reps - 1)
            mark(f"D{reps-1}")
            emit_D(c, state, reps - 1)
            mark("end")

    nc.finalize()
    return nc


def emit_A(c, state, rep):
    """Projections Vq/Vk/Vv + Vv transpose; prefetch Wo/residual."""
    nc, tc = c.nc, c.tc
    s = state[rep] = {}
    s["vq"] = c.persist.tile([128, S], F16, tag="vq", name=f"vq_{rep}")
    s["vk"] = c.persist.tile([128, S], F16, tag="vk", name=f"vk_{rep}")
    s["vvT"] = c.persist.tile([128, TT, 2 * (HD + 1)], BF16, tag="vvT", name=f"vvT_{rep}")
    s["heads"] = c.persist.tile([128, S], F16, tag="heads", name=f"heads_{rep}")
    s["wo"] = c.persist.tile([128, KT, D], F16, tag="wo", name=f"wo_{rep}")
    s["vs"] = c.persist.tile([128, 2, D], F32, tag="vs", name=f"vs_{rep}")

    wq_sb = c.projw.tile([128, KT, 128], F16, tag="wq", name=f"wq_{rep}")
    wk_sb = c.projw.tile([128, KT, 128], F16, tag="wk", name=f"wk_{rep}")
    wv_sb = c.projw.tile([128, KT, 128], BF16, tag="wv", name=f"wv_{rep}")
    nc.sync.dma_start(wk_sb[:], c.wkT[:])
    nc.sync.dma_start(wq_sb[:], c.wqT[:])
    nc.sync.dma_start(wv_sb[:], c.wvT[:])
    vv_sb = c.projw.tile([128, S], BF16, tag="vv", name=f"vv_{rep}")

    with (
        tc.tile_pool(name=f"projps_{rep}", bufs=1, space="PSUM") as projps,
        tc.tile_pool(name=f"trps_{rep}", bufs=2, space="PSUM") as trps,
    ):
        for (src, wsb, dst, xdt) in ((c.Kt, wk_sb, s["vk"], F16),
                                     (c.Qt, wq_sb, s["vq"], F16),
                                     (c.Vt, wv_sb, vv_sb, BF16)):
            ps = projps.tile([128, S], F32, tag="proj_ps", name=f"proj_ps_{rep}")
            for k in range(KT):
                xin = c.projin.tile([128, S], xdt, tag="proj_in",
                                    name=f"proj_in_{rep}_{k}")
                nc.sync.dma_start(xin[:], src.ap()[k])
                for j in range(NSCH):
                    nc.tensor.matmul(
                        ps[:, j * SCH:(j + 1) * SCH],
                        wsb[:, k, :], xin[:, j * SCH:(j + 1) * SCH],
                        start=(k == 0), stop=(k == KT - 1),
                    )
            for j in range(NSCH):
                nc.vector.tensor_copy(dst[:, j * SCH:(j + 1) * SCH],
                                      ps[:, j * SCH:(j + 1) * SCH])

        # prefetch Wo + residual behind the projection-input DMAs
        nc.sync.dma_start(s["wo"][:], c.Wot[:])
        nc.sync.dma_start(s["vs"][:], c.VsT.ap().rearrange("st p d -> p st d"))

        # transpose Vv [(h d), t] -> vvT [t, (d|1)*2] per t_tile, with ones col
        nc.gpsimd.memset(s["vvT"][:], 1.0)  # ones columns come for free
        for t in range(TT):
            pst = trps.tile([128, 128], BF16, tag="tr_ps", name=f"trps_{rep}_{t}")
            nc.tensor.transpose(pst[:], vv_sb[:, t * 128:(t + 1) * 128], c.ident[:])
            nc.vector.tensor_copy(s["vvT"][:, t, 0:HD], pst[:, 0:HD])
            nc.vector.tensor_copy(s["vvT"][:, t, HD + 1:2 * HD + 1], pst[:, HD:2 * HD])


def emit_B(c, state, rep):
    """Attention per head; per-head A2A overlapped with the next head."""
    nc, tc = c.nc, c.tc
    s = state[rep]
    heads_sb = s["heads"]
    s["sends"] = [c.dram.tile([NCORES, HD, SL], F16, tag=f"send{h}",
                              name=f"send_{rep}_{h}") for h in range(HPC)]
    s["recvs"] = [c.dram.tile([NCORES, HD, SL], F16, tag=f"recv{h}",
                              name=f"recv_{rep}_{h}") for h in range(HPC)]
    with (
        tc.tile_pool(name=f"scps_{rep}", bufs=2, space="PSUM") as scps,
        tc.tile_pool(name=f"avps_{rep}", bufs=4, space="PSUM") as avps,
    ):
        for h in range(HPC):
            hp = h * 64        # partition offset of this head in vq/vk
            lo = h * (HD + 1)  # free offset of this head (+ones) in vvT
            ps_h = [avps.tile([128, SCH], F32, tag="av_ps", name=f"av_ps_{rep}_{h}_{j}")
                    for j in range(NSCH)]

            def _scores(t):
                e_t = c.esb.tile([128, S], BF16, tag="e", name=f"e_{rep}_{h}_{t}")
                jd = t // 4  # j-chunk containing this t-tile's diagonal block
                for half in range(2):
                    ps_s = scps.tile([128, 2 * SCH], F32, tag="sc_ps",
                                     name=f"sc_ps_{rep}_{h}_{t}_{half}")
                    for jj in range(2):
                        j = half * 2 + jj
                        masked = (j == jd)
                        nc.tensor.matmul(
                            ps_s[:, jj * SCH:(jj + 1) * SCH],
                            s["vk"][hp:hp + 64, t * 128:(t + 1) * 128],
                            s["vq"][hp:hp + 64, j * SCH:(j + 1) * SCH],
                            start=True, stop=not masked,
                        )
                        if masked:
                            off = jj * SCH + (t * 128 - jd * SCH)
                            nc.tensor.matmul(
                                ps_s[:, off:off + 128],
                                c.mdiag[:], c.ident[:],
                                start=False, stop=True,
                            )
                    # E = exp(scores / 8), one ACT op over both banks
                    nc.scalar.activation(
                        e_t[:, half * 2 * SCH:(half + 1) * 2 * SCH],
                        ps_s[:], AF.Exp, scale=ISCALE)
                return e_t

            def _av(t, e_t):
                for j in range(NSCH):
                    nc.tensor.matmul(
                        ps_h[j][0:HD + 1, :],
                        s["vvT"][:, t, lo:lo + HD + 1],
                        e_t[:, j * SCH:(j + 1) * SCH],
                        start=(t == 0), stop=(t == TT - 1),
                    )

            # software-pipelined: scores(t+1) issues on the PE before AV(t),
            # so the exp(t) latency on ACT hides behind score matmuls
            prev = None
            for t in range(TT):
                e_t = _scores(t)
                if prev is not None:
                    _av(*prev)
                prev = (t, e_t)
            _av(*prev)
            for j in range(NSCH):
                rec = c.avsb.tile([1, SCH], F32, tag="av_rec", name=f"rec_{rep}_{h}_{j}")
                nc.vector.reciprocal(rec[:], ps_h[j][HD:HD + 1, :])
                rb = c.avsb.tile([HD, SCH], F32, tag="av_rb", name=f"rb_{rep}_{h}_{j}")
                nc.gpsimd.partition_broadcast(rb[:], rec[:])
                nc.vector.tensor_mul(
                    out=heads_sb[hp:hp + 64, j * SCH:(j + 1) * SCH],
                    in0=ps_h[j][0:HD, :], in1=rb[:])
            # stage + exchange this head's block while the next head computes
            nc.sync.dma_start(
                s["sends"][h].rearrange("d p s -> p d s"),
                heads_sb[hp:hp + 64, :].rearrange("p (d s) -> p d s", d=NCORES))
            if os.environ.get("KERNEL_NO_CC"):
                nc.sync.dma_start(s["recvs"][h][:], s["sends"][h][:])  # timing-only
            else:
                nc.gpsimd.collective_compute(
                    "AllToAll", mybir.AluOpType.bypass,
                    replica_groups=[list(range(NCORES))],
                    ins=[s["sends"][h].opt()], outs=[s["recvs"][h].opt()],
                )


def emit_C(c, state, rep):
    """Wo projection, residual + LN1, transpose for the FFN."""
    nc, tc = c.nc, c.tc
    s = state[rep]
    s["z"] = c.persist.tile([128, 2, D], F32, tag="z", name=f"z_{rep}")
    s["xT"] = c.persist.tile([128, KT, SL], F16, tag="xT", name=f"xT_{rep}")
    recvT = c.persist.tile([128, NCORES, SL], F16, tag="recvT", name=f"recvT_{rep}")
    for h in range(HPC):
        nc.sync.dma_start(recvT[h * HD:(h + 1) * HD, :, :],
                          s["recvs"][h].rearrange("j p s -> p j s"))
    with (
        tc.tile_pool(name=f"wops_{rep}", bufs=2, space="PSUM") as wops,
        tc.tile_pool(name=f"trps2_{rep}", bufs=2, space="PSUM") as trps2,
    ):
        # both Wo st-tiles first (PE stays busy while LN1(st0) runs on DVE),
        # then LN(st) -> transposes(st) interleaved
        x_sbs = []
        for st in range(2):  # two tiles of 128 seq positions
            x_sb = c.lnsb.tile([128, D], F32, tag=f"x1_{st}", name=f"x1_{rep}_{st}")
            x_sbs.append(x_sb)
            for nchunk in range(2):
                ps_o = wops.tile([128, SCH], F32, tag="wo_ps",
                                 name=f"wo_ps_{rep}_{st}_{nchunk}")
                for k in range(KT):
                    nc.tensor.matmul(
                        ps_o[:],
                        recvT[:, k, st * 128:(st + 1) * 128],
                        s["wo"][:, k, nchunk * SCH:(nchunk + 1) * SCH],
                        start=(k == 0), stop=(k == KT - 1),
                    )
                nc.vector.tensor_add(
                    out=x_sb[:, nchunk * SCH:(nchunk + 1) * SCH],
                    in0=ps_o[:],
                    in1=s["vs"][:, st, nchunk * SCH:(nchunk + 1) * SCH],
                )
        # critical path: normalized u = (x-mu)*rec feeds the transposes/FFN1
        # directly — the *a2+b2 affine is folded into W1'/b1' on the host.
        for st in range(2):
            _ln_norm(nc, c.lnsb, x_sbs[st], f"{rep}_1_{st}")
            for dt in range(KT):
                pst = trps2.tile([128, 128], F32, tag="tr2_ps", name=f"tr2_{rep}_{st}_{dt}")
                nc.tensor.transpose(pst[:], x_sbs[st][:, dt * 128:(dt + 1) * 128],
                                    c.identf[:])
                nc.vector.tensor_copy(s["xT"][:, dt, st * 128:(st + 1) * 128], pst[:])
        # full z (u*a2 + b2) only gates the FFN2 residual — off the critical path
        for st in range(2):
            nc.vector.scalar_tensor_tensor(
                out=s["z"][:, st, :], in0=x_sbs[st][:], scalar=1.0,
                in1=c.a2_sb[:], op0=mybir.AluOpType.mult, op1=mybir.AluOpType.mult)
            nc.vector.tensor_add(out=s["z"][:, st, :], in0=s["z"][:, st, :],
                                 in1=c.b2n_sb[:])


def emit_D(c, state, rep):
    """FFN (streamed W1/W2) + residual + LN2 + output DMA."""
    nc, tc = c.nc, c.tc
    s = state[rep]
    ffh_sb = c.persist.tile([128, FFT, SL], F16, tag="ffh", name=f"ffh_{rep}")
    with (
        tc.tile_pool(name=f"ffps_{rep}", bufs=4, space="PSUM") as ffps,
        tc.tile_pool(name=f"ff2ps_{rep}", bufs=1, space="PSUM") as ff2ps,
    ):
        for g in range(8):
            w1_sb = c.w1s.tile([128, 4, KT, 128], F16, tag="w1", name=f"w1_{rep}_{g}")
            nc.sync.dma_start(w1_sb[:], c.W1g[g])
            for mi in range(4):
                m = g * 4 + mi
                ps_f = ffps.tile([128, SL], F32, tag="ff_ps", name=f"ff_ps_{rep}_{m}")
                for k in range(KT):
                    nc.tensor.matmul(
                        ps_f[:], w1_sb[:, mi, k, :], s["xT"][:, k, :],
                        start=(k == 0), stop=(k == KT - 1),
                    )
                nc.scalar.activation(ffh_sb[:, m, :], ps_f[:], AF.Relu,
                                     bias=c.b1_sb[:, m:m + 1], scale=1.0)

        # swapped FFN2: psum [s, d-chunk] = ffh_tile.T @ W2T_tile
        ps_g = [ff2ps.tile([128, SCH], F32, tag=f"ff2_ps{i}", name=f"ff2_ps_{rep}_{i}")
                for i in range(4)]
        for g in range(8):
            w2_sb = c.w2s.tile([128, 4, D], F16, tag="w2", name=f"w2_{rep}_{g}")
            nc.sync.dma_start(w2_sb[:], c.W2g[g])
            for ki in range(4):
                k = g * 4 + ki
                for st in range(2):
                    for dc in range(2):
                        nc.tensor.matmul(
                            ps_g[st * 2 + dc][:],
                            ffh_sb[:, k, st * 128:(st + 1) * 128],
                            w2_sb[:, ki, dc * SCH:(dc + 1) * SCH],
                            start=(k == 0), stop=(k == FFT - 1),
                        )
        for st in range(2):
            x2_sb = c.lnsb.tile([128, D], F32, tag="x2", name=f"x2_{rep}_{st}")
            for dc in range(2):
                nc.vector.tensor_add(
                    out=x2_sb[:, dc * SCH:(dc + 1) * SCH],
                    in0=ps_g[st * 2 + dc][:],
                    in1=s["z"][:, st, dc * SCH:(dc + 1) * SCH],
                )
            nc.vector.tensor_add(out=x2_sb[:], in0=x2_sb[:], in1=c.b2f_sb[:])
            _ln(nc, c.lnsb, x2_sb, x2_sb, c.a2_sb, c.b2n_sb, f"{rep}_2_{st}")
            nc.sync.dma_start(c.outs[rep].ap()[st], x2_sb[:])


_NC_CACHE = {}


def _get_nc(reps: int = 1):
    if reps not in _NC_CACHE:
        _NC_CACHE[reps] = build(reps)
    return _NC_CACHE[reps]


def prep_inputs(Q, K, V, wq, wk, wv, Wo, W1, b1, W2, b2, a_2, b_2):
    """Host-side sharding/layout prep. Returns per-core input maps."""
    f32 = np.float32
    bf = BF16NP
    Q = np.asarray(Q, f32); K = np.asarray(K, f32); V = np.asarray(V, f32)
    Qt = np.ascontiguousarray(Q.reshape(KT, 128, S).astype(F16NP))
    Kt = np.ascontiguousarray(K.reshape(KT, 128, S).astype(F16NP))
    Vt = np.ascontiguousarray(V.reshape(KT, 128, S).astype(bf))
    Wot = np.ascontiguousarray(
        np.asarray(Wo, f32).reshape(KT, 128, D).transpose(1, 0, 2).astype(F16NP))
    # FFN1 consumes the pre-affine normalized activations: fold LN1's affine
    # into the weights/bias: W1' = W1 * a2 (per input column), b1' = b1 + W1 @ b2
    W1p = np.asarray(W1, f32) * np.asarray(a_2, f32)[None, :]
    b1 = np.asarray(b1, f32) + np.asarray(W1, f32) @ np.asarray(b_2, f32)
    # W1g: [8, 128(ki), 4(m-sub), 8(kt), 128(m)]
    W1g = np.ascontiguousarray(
        W1p.reshape(8, 4, 128, KT, 128)  # [g, msub, m, kt, ki]
        .transpose(0, 4, 1, 3, 2).astype(F16NP))             # -> [g, ki, msub, kt, m]
    # W2g: [8, 128(ki over f), 4(kt-sub over f), 1024(d)]
    W2g = np.ascontiguousarray(
        np.asarray(W2, f32).T.reshape(8, 4, 128, D).transpose(0, 2, 1, 3).astype(F16NP))
    b1t = np.ascontiguousarray(np.asarray(b1, f32).reshape(FFT, 128).T)
    b2vr = np.asarray(b2, f32).reshape(1, D)
    a2vr = np.asarray(a_2, f32).reshape(1, D)
    b2nvr = np.asarray(b_2, f32).reshape(1, D)

    wq = np.asarray(wq, f32); wk = np.asarray(wk, f32); wv = np.asarray(wv, f32)
    in_maps = []
    for c in range(NCORES):
        def _wT(w, dt=bf):
            wc = w[c * HPC:(c + 1) * HPC].reshape(128, D)  # [m, k]
            return np.ascontiguousarray(
                wc.reshape(128, KT, 128).transpose(2, 1, 0).astype(dt))
        VsT = np.ascontiguousarray(V[:, c * SL:(c + 1) * SL].T.reshape(2, 128, D))
        in_maps.append({
            "Qt": Qt, "Kt": Kt, "Vt": Vt,
            "wqT": _wT(wq, F16NP), "wkT": _wT(wk, F16NP), "wvT": _wT(wv),
            "Wot": Wot, "W1g": W1g, "W2g": W2g,
            "b1t": b1t, "b2v": b2vr, "a2v": a2vr, "b2nv": b2nvr,
            "VsT": VsT,
        })
    return in_maps


def run(in_maps, reps: int = 1):
    nc = _get_nc(reps)
    return run_bass_kernel_spmd(nc, in_maps, list(range(NCORES)))


def assemble(results, rep=0):
    """[2,128,1024] per core -> full [1024, 2048] output."""
    z2 = np.concatenate(
        [results[c][f"out{rep}"].reshape(2 * 128, D) for c in range(NCORES)], axis=0)
    return np.ascontiguousarray(z2.T)


def kernel(Q, K, V, wq, wk, wv, Wo, W1, b1, W2, b2, a_2, b_2):
    in_maps = prep_inputs(Q, K, V, wq, wk, wv, Wo, W1, b1, W2, b2, a_2, b_2)
    res = run(in_maps, reps=1).results
    return assemble(res)


# revision 48
# speedup vs baseline: 1.5905x; 1.5905x over previous
"""Trainium2 Bass kernel for nn_EncoderLayer (D=1024, H=16, S=2048, FF=4096), 8-core SPMD.

Strategy: head-parallel attention (2 heads/core), fp16 AllToAll per head to
switch to sequence-parallel (256 positions/core) for the output projection,
norms and FFN. No all-reduce needed anywhere.

Precision: fp16 matmul datapath (1 PE cycle/row, fp32 PSUM accumulation,
fp32 LayerNorm/softmax arithmetic). Only exp(scores) and the V/AV path use
bf16 (e values reach ~e^20, past fp16 range). rel_err vs fp32 ref ~3e-3.

Schedule: per-head A2A overlapped with the other head's compute; the
diagonal mask is matmul-accumulated (-1e5*I) into the score PSUM so no
gpsimd select sits in the exp->AV chain; the t-loop is software-pipelined
(scores(t+1) issues before AV(t)); LayerNorm uses bn_stats + fused
tensor_scalar ops; bodies are software-pipelined across reps: phase A of
body r+1 is emitted inside body r's A2A window so the PE never drains.
"""
import math
import os

import numpy as np
import ml_dtypes

import concourse.bass as bass
import concourse.mybir as mybir
import concourse.tile as tile
from concourse import bacc
from concourse.bass_utils import run_bass_kernel_spmd
from concourse.masks import make_identity

F32 = mybir.dt.float32
F32R = mybir.dt.float32r  # TF32-like PE mode: 1 cycle/row for >=256-row moving ops
F16 = mybir.dt.float16    # same speed/bytes as bf16, 3 more mantissa bits
F16NP = np.float16
BF16 = mybir.dt.bfloat16
AF = mybir.ActivationFunctionType
BF16NP = ml_dtypes.bfloat16

D = 1024
H = 16
HD = 64
S = 2048
FF = 4096
EPS = 1e-3
NCORES = 8
SL = S // NCORES          # 256 sequence positions per core after A2A
HPC = H // NCORES         # 2 heads per core
KT = D // 128             # 8 k-tiles over the model dim
TT = S // 128             # 16 t-tiles over sequence
SCH = 512                 # PSUM-bank-limited matmul output chunk
NSCH = S // SCH           # 4 s-chunks
FFT = FF // 128           # 32 hidden tiles
ISCALE = 1.0 / math.sqrt(HD)


def _ln_norm(nc, pools, x_sb, tag):
    """In-place x <- (x - mu) / (std_ddof1 + eps) over the free axis (1024).

    bn_stats gives mean+variance in one pass (two 512 groups + aggregate);
    the ddof=1 correction folds into the sqrt's scale.
    """
    stats = pools.tile([128, 2, 6], F32, tag="ln_st", name=f"ln_st_{tag}")
    for g in range(2):
        nc.vector.bn_stats(stats[:, g, :], x_sb[:, g * 512:(g + 1) * 512])
    mv = pools.tile([128, 2], F32, tag="ln_mv", name=f"ln_mv_{tag}")
    nc.vector.bn_aggr(mv[:], stats[:])
    sig = pools.tile([128, 1], F32, tag="ln_sig", name=f"ln_sig_{tag}")
    # sigma_ddof1 = sqrt(var * D/(D-1)); then += eps; then reciprocal
    nc.scalar.activation(sig[:], mv[:, 1:2], AF.Sqrt, scale=float(D) / (D - 1))
    nc.vector.tensor_scalar_add(sig[:], sig[:], EPS)
    rec = pools.tile([128, 1], F32, tag="ln_rec", name=f"ln_rec_{tag}")
    nc.vector.reciprocal(rec[:], sig[:])
    # in-place: x_sb <- (x - mu) * rec
    nc.vector.tensor_scalar(out=x_sb[:], in0=x_sb[:], scalar1=mv[:, 0:1],
                            scalar2=rec[:], op0=mybir.AluOpType.subtract,
                            op1=mybir.AluOpType.mult)


def _ln(nc, pools, x_sb, z_sb, a2_sb, b2n_sb, tag):
    """Full LayerNorm: z = (x - mu)/(std_ddof1 + eps) * a2 + b2."""
    _ln_norm(nc, pools, x_sb, tag)
    nc.vector.scalar_tensor_tensor(out=z_sb[:], in0=x_sb[:], scalar=1.0,
                                   in1=a2_sb[:], op0=mybir.AluOpType.mult,
                                   op1=mybir.AluOpType.mult)
    nc.vector.tensor_add(out=z_sb[:], in0=z_sb[:], in1=b2n_sb[:])


class _Ctx:
    """Program-level pools + parameters shared by all pipelined bodies."""


PHASE_MARKS = []  # (first instruction id, phase label) — for sim profiling


def build(reps: int = 1):
    nc = bacc.Bacc("TRN2", target_bir_lowering=False, debug=False, num_devices=NCORES)
    c = _Ctx()
    c.nc = nc

    # ---- DRAM parameters (per-core shards prepared on host, bf16 datapath) ----
    c.Qt = nc.declare_dram_parameter("Qt", [KT, 128, S], F16, isOutput=False)
    c.Kt = nc.declare_dram_parameter("Kt", [KT, 128, S], F16, isOutput=False)
    c.Vt = nc.declare_dram_parameter("Vt", [KT, 128, S], BF16, isOutput=False)
    c.wqT = nc.declare_dram_parameter("wqT", [128, KT, 128], F16, isOutput=False)
    c.wkT = nc.declare_dram_parameter("wkT", [128, KT, 128], F16, isOutput=False)
    c.wvT = nc.declare_dram_parameter("wvT", [128, KT, 128], BF16, isOutput=False)
    c.Wot = nc.declare_dram_parameter("Wot", [128, KT, D], F16, isOutput=False)
    # W1g[g] : [128(ki), 4(m-sub), 8(kt), 128(m)]  contiguous 1MB bf16 blocks
    c.W1g = nc.declare_dram_parameter("W1g", [8, 128, 4, KT, 128], F16, isOutput=False)
    # W2g[g] : [128(ki over f), 4(kt-sub over f), 1024(d)] contiguous 1MB blocks
    c.W2g = nc.declare_dram_parameter("W2g", [8, 128, 4, D], F16, isOutput=False)
    c.b1t = nc.declare_dram_parameter("b1t", [128, FFT], F32, isOutput=False)
    b2v = nc.declare_dram_parameter("b2v", [1, D], F32, isOutput=False)
    a2v = nc.declare_dram_parameter("a2v", [1, D], F32, isOutput=False)
    b2nv = nc.declare_dram_parameter("b2nv", [1, D], F32, isOutput=False)
    c.VsT = nc.declare_dram_parameter("VsT", [2, 128, D], F32, isOutput=False)
    # all reps write the same output buffer: extra reps add zero host
    # transfer, so reps-diff timing isolates device-side work
    out0 = nc.declare_dram_parameter("out0", [2, 128, D], F32, isOutput=True)
    c.outs = [out0] * reps

    with tile.TileContext(nc) as tc:
        c.tc = tc
        import contextlib
        with contextlib.ExitStack() as st:
            ent = st.enter_context
            c.singles = ent(tc.tile_pool(name="singles", bufs=1))
            c.persist = ent(tc.tile_pool(name="persist", bufs=1))
            c.projw = ent(tc.tile_pool(name="projw", bufs=1))
            c.projin = ent(tc.tile_pool(name="projin", bufs=8))
            c.esb = ent(tc.tile_pool(name="esb", bufs=3))
            c.avsb = ent(tc.tile_pool(name="avsb", bufs=2))
            c.lnsb = ent(tc.tile_pool(name="lnsb", bufs=1))
            c.w1s = ent(tc.tile_pool(name="w1s", bufs=2))
            c.w2s = ent(tc.tile_pool(name="w2s", bufs=2))
            c.dram = ent(tc.tile_pool(name="dram", bufs=2, space="DRAM"))

            ident = c.singles.tile([128, 128], BF16)
            make_identity(nc, ident[:])
            identf = c.singles.tile([128, 128], F32)
            make_identity(nc, identf[:])
            # -1e5 * I — matmul-accumulated into the score PSUM to apply the
            # diagonal attention mask without a gpsimd select in the chain
            mdiag = c.singles.tile([128, 128], BF16)
            nc.gpsimd.memset(mdiag[:], 0.0)
            nc.gpsimd.affine_select(
                out=mdiag[:], in_=mdiag[:],
                compare_op=mybir.AluOpType.not_equal,
                fill=-100000.0, base=0,
                pattern=[[-1, 128]], channel_multiplier=1,
            )
            c.mdiag = mdiag
            c.ident, c.identf = ident, identf
            c.a2_sb = c.singles.tile([128, D], F32)
            c.b2n_sb = c.singles.tile([128, D], F32)
            c.b2f_sb = c.singles.tile([128, D], F32)
            c.b1_sb = c.singles.tile([128, FFT], F32)
            vrow = c.singles.tile([1, D], F32)
            nc.sync.dma_start(vrow[:], a2v[:])
            nc.gpsimd.partition_broadcast(c.a2_sb[:], vrow[:])
            nc.sync.dma_start(vrow[:], b2nv[:])
            nc.gpsimd.partition_broadcast(c.b2n_sb[:], vrow[:])
            nc.sync.dma_start(vrow[:], b2v[:])
            nc.gpsimd.partition_broadcast(c.b2f_sb[:], vrow[:])
            nc.sync.dma_start(c.b1_sb[:], c.b1t[:])

            # Pipelined emission: A(0) B(0) | A(1) C(0) D(0) B(1) | A(2) C(1) ...
            # Body r+1's projections run inside body r's A2A window.
            def mark(label):
                PHASE_MARKS.append((int(nc.next_id()), label))

            PHASE_MARKS.clear()
            state = {}
            mark("A0")
            emit_A(c, state, 0)
            mark("B0")
            emit_B(c, state, 0)
            for r in range(1, reps):
                mark(f"A{r}")
                emit_A(c, state, r)
                mark(f"C{r-1}")
                emit_C(c, state, r - 1)
                mark(f"D{r-1}")
                emit_D(c, state, r - 1)
                mark(f"B{r}")
                emit_B(c, state, r)
            mark(f"C{reps-1}")
            emit_C(c, state, reps - 1)
            mark(f"D{reps-1}")
            emit_D(c, state, reps - 1)
            mark("end")

    nc.finalize()
    return nc


def emit_A(c, state, rep):
    """Projections Vq/Vk/Vv + Vv transpose; prefetch Wo/residual."""
    nc, tc = c.nc, c.tc
    s = state[rep] = {}
    s["vq"] = c.persist.tile([128, S], F16, tag="vq", name=f"vq_{rep}")
    s["vk"] = c.persist.tile([128, S], F16, tag="vk", name=f"vk_{rep}")
    s["vvT"] = c.persist.tile([128, TT, 2 * (HD + 1)], BF16, tag="vvT", name=f"vvT_{rep}")
    s["heads"] = c.persist.tile([128, S], F16, tag="heads", name=f"heads_{rep}")
    s["wo"] = c.persist.tile([128, KT, D], F16, tag="wo", name=f"wo_{rep}")
    s["vs"] = c.persist.tile([128, 2, D], F32, tag="vs", name=f"vs_{rep}")

    wq_sb = c.projw.tile([128, KT, 128], F16, tag="wq", name=f"wq_{rep}")
    wk_sb = c.projw.tile([128, KT, 128], F16, tag="wk", name=f"wk_{rep}")
    wv_sb = c.projw.tile([128, KT, 128], BF16, tag="wv", name=f"wv_{rep}")
    nc.sync.dma_start(wk_sb[:], c.wkT[:])
    nc.sync.dma_start(wq_sb[:], c.wqT[:])
    nc.sync.dma_start(wv_sb[:], c.wvT[:])
    vv_sb = c.projw.tile([128, S], BF16, tag="vv", name=f"vv_{rep}")

    with (
        tc.tile_pool(name=f"projps_{rep}", bufs=1, space="PSUM") as projps,
        tc.tile_pool(name=f"trps_{rep}", bufs=2, space="PSUM") as trps,
    ):
        for (src, wsb, dst, xdt) in ((c.Kt, wk_sb, s["vk"], F16),
                                     (c.Vt, wv_sb, vv_sb, BF16),
                                     (c.Qt, wq_sb, s["vq"], F16)):
            ps = projps.tile([128, S], F32, tag="proj_ps", name=f"proj_ps_{rep}")
            for k in range(KT):
                xin = c.projin.tile([128, S], xdt, tag="proj_in",
                                    name=f"proj_in_{rep}_{k}")
                nc.sync.dma_start(xin[:], src.ap()[k])
                for j in range(NSCH):
                    nc.tensor.matmul(
                        ps[:, j * SCH:(j + 1) * SCH],
                        wsb[:, k, :], xin[:, j * SCH:(j + 1) * SCH],
                        start=(k == 0), stop=(k == KT - 1),
                    )
            for j in range(NSCH):
                nc.vector.tensor_copy(dst[:, j * SCH:(j + 1) * SCH],
                                      ps[:, j * SCH:(j + 1) * SCH])

        # prefetch Wo + residual behind the projection-input DMAs
        nc.sync.dma_start(s["wo"][:], c.Wot[:])
        nc.sync.dma_start(s["vs"][:], c.VsT.ap().rearrange("st p d -> p st d"))

        # transpose Vv [(h d), t] -> vvT [t, (d|1)*2] per t_tile, with ones col
        nc.gpsimd.memset(s["vvT"][:], 1.0)  # ones columns come for free
        for t in range(TT):
            pst = trps.tile([128, 128], BF16, tag="tr_ps", name=f"trps_{rep}_{t}")
            nc.tensor.transpose(pst[:], vv_sb[:, t * 128:(t + 1) * 128], c.ident[:])
            nc.vector.tensor_copy(s["vvT"][:, t, 0:HD], pst[:, 0:HD])
            nc.vector.tensor_copy(s["vvT"][:, t, HD + 1:2 * HD + 1], pst[:, HD:2 * HD])


def emit_B(c, state, rep):
    """Attention per head; per-head A2A overlapped with the next head."""
    nc, tc = c.nc, c.tc
    s = state[rep]
    heads_sb = s["heads"]
    s["sends"] = [c.dram.tile([NCORES, HD, SL], F16, tag=f"send{h}",
                              name=f"send_{rep}_{h}") for h in range(HPC)]
    s["recvs"] = [c.dram.tile([NCORES, HD, SL], F16, tag=f"recv{h}",
                              name=f"recv_{rep}_{h}") for h in range(HPC)]
    with (
        tc.tile_pool(name=f"scps_{rep}", bufs=2, space="PSUM") as scps,
        tc.tile_pool(name=f"avps_{rep}", bufs=4, space="PSUM") as avps,
    ):
        for h in range(HPC):
            hp = h * 64        # partition offset of this head in vq/vk
            lo = h * (HD + 1)  # free offset of this head (+ones) in vvT
            ps_h = [avps.tile([128, SCH], F32, tag="av_ps", name=f"av_ps_{rep}_{h}_{j}")
                    for j in range(NSCH)]

            def _scores(t):
                e_t = c.esb.tile([128, S], BF16, tag="e", name=f"e_{rep}_{h}_{t}")
                jd = t // 4  # j-chunk containing this t-tile's diagonal block
                for half in range(2):
                    ps_s = scps.tile([128, 2 * SCH], F32, tag="sc_ps",
                                     name=f"sc_ps_{rep}_{h}_{t}_{half}")
                    for jj in range(2):
                        j = half * 2 + jj
                        masked = (j == jd)
                        nc.tensor.matmul(
                            ps_s[:, jj * SCH:(jj + 1) * SCH],
                            s["vk"][hp:hp + 64, t * 128:(t + 1) * 128],
                            s["vq"][hp:hp + 64, j * SCH:(j + 1) * SCH],
                            start=True, stop=not masked,
                        )
                        if masked:
                            off = jj * SCH + (t * 128 - jd * SCH)
                            nc.tensor.matmul(
                                ps_s[:, off:off + 128],
                                c.mdiag[:], c.ident[:],
                                start=False, stop=True,
                            )
                    # E = exp(scores / 8), one ACT op over both banks
                    nc.scalar.activation(
                        e_t[:, half * 2 * SCH:(half + 1) * 2 * SCH],
                        ps_s[:], AF.Exp, scale=ISCALE)
                return e_t

            def _av(t, e_t):
                for j in range(NSCH):
                    nc.tensor.matmul(
                        ps_h[j][0:HD + 1, :],
                        s["vvT"][:, t, lo:lo + HD + 1],
                        e_t[:, j * SCH:(j + 1) * SCH],
                        start=(t == 0), stop=(t == TT - 1),
                    )

            # software-pipelined: scores(t+1) issues on the PE before AV(t),
            # so the exp(t) latency on ACT hides behind score matmuls
            prev = None
            for t in range(TT):
                e_t = _scores(t)
                if prev is not None:
                    _av(*prev)
                prev = (t, e_t)
            _av(*prev)
            for j in range(NSCH):
                rec = c.avsb.tile([1, SCH], F32, tag="av_rec", name=f"rec_{rep}_{h}_{j}")
                nc.vector.reciprocal(rec[:], ps_h[j][HD:HD + 1, :])
                rb = c.avsb.tile([HD, SCH], F32, tag="av_rb", name=f"rb_{rep}_{h}_{j}")
                nc.gpsimd.partition_broadcast(rb[:], rec[:])
                nc.vector.tensor_mul(
                    out=heads_sb[hp:hp + 64, j * SCH:(j + 1) * SCH],
                    in0=ps_h[j][0:HD, :], in1=rb[:])
            # stage + exchange this head's block while the next head computes
            nc.sync.dma_start(
                s["sends"][h].rearrange("d p s -> p d s"),
                heads_sb[hp:hp + 64, :].rearrange("p (d s) -> p d s", d=NCORES))
            if os.environ.get("KERNEL_NO_CC"):
                nc.sync.dma_start(s["recvs"][h][:], s["sends"][h][:])  # timing-only
            else:
                nc.gpsimd.collective_compute(
                    "AllToAll", mybir.AluOpType.bypass,
                    replica_groups=[list(range(NCORES))],
                    ins=[s["sends"][h].opt()], outs=[s["recvs"][h].opt()],
                )


def emit_C(c, state, rep):
    """Wo projection, residual + LN1, transpose for the FFN."""
    nc, tc = c.nc, c.tc
    s = state[rep]
    s["z"] = c.persist.tile([128, 2, D], F32, tag="z", name=f"z_{rep}")
    s["xT"] = c.persist.tile([128, KT, SL], F16, tag="xT", name=f"xT_{rep}")
    recvT = c.persist.tile([128, NCORES, SL], F16, tag="recvT", name=f"recvT_{rep}")
    for h in range(HPC):
        nc.sync.dma_start(recvT[h * HD:(h + 1) * HD, :, :],
                          s["recvs"][h].rearrange("j p s -> p j s"))
    with (
        tc.tile_pool(name=f"wops_{rep}", bufs=2, space="PSUM") as wops,
        tc.tile_pool(name=f"trps2_{rep}", bufs=2, space="PSUM") as trps2,
    ):
        # both Wo st-tiles first (PE stays busy while LN1(st0) runs on DVE),
        # then LN(st) -> transposes(st) interleaved
        x_sbs = []
        for st in range(2):  # two tiles of 128 seq positions
            x_sb = c.lnsb.tile([128, D], F32, tag=f"x1_{st}", name=f"x1_{rep}_{st}")
            x_sbs.append(x_sb)
            for nchunk in range(2):
                ps_o = wops.tile([128, SCH], F32, tag="wo_ps",
                                 name=f"wo_ps_{rep}_{st}_{nchunk}")
                for k in range(KT):
                    nc.tensor.matmul(
                        ps_o[:],
                        recvT[:, k, st * 128:(st + 1) * 128],
                        s["wo"][:, k, nchunk * SCH:(nchunk + 1) * SCH],
                        start=(k == 0), stop=(k == KT - 1),
                    )
                nc.vector.tensor_add(
                    out=x_sb[:, nchunk * SCH:(nchunk + 1) * SCH],
                    in0=ps_o[:],
                    in1=s["vs"][:, st, nchunk * SCH:(nchunk + 1) * SCH],
                )
        # critical path: normalized u = (x-mu)*rec feeds the transposes/FFN1
        # directly — the *a2+b2 affine is folded into W1'/b1' on the host.
        for st in range(2):
            _ln_norm(nc, c.lnsb, x_sbs[st], f"{rep}_1_{st}")
            for dt in range(KT):
                pst = trps2.tile([128, 128], F32, tag="tr2_ps", name=f"tr2_{rep}_{st}_{dt}")
                nc.tensor.transpose(pst[:], x_sbs[st][:, dt * 128:(dt + 1) * 128],
                                    c.identf[:])
                nc.vector.tensor_copy(s["xT"][:, dt, st * 128:(st + 1) * 128], pst[:])
        # full z (u*a2 + b2) only gates the FFN2 residual — off the critical path
        for st in range(2):
            nc.vector.scalar_tensor_tensor(
                out=s["z"][:, st, :], in0=x_sbs[st][:], scalar=1.0,
                in1=c.a2_sb[:], op0=mybir.AluOpType.mult, op1=mybir.AluOpType.mult)
            nc.vector.tensor_add(out=s["z"][:, st, :], in0=s["z"][:, st, :],
                                 in1=c.b2n_sb[:])


def emit_D(c, state, rep):
    """FFN (streamed W1/W2) + residual + LN2 + output DMA."""
    nc, tc = c.nc, c.tc
    s = state[rep]
    ffh_sb = c.persist.tile([128, FFT, SL], F16, tag="ffh", name=f"ffh_{rep}")
    with (
        tc.tile_pool(name=f"ffps_{rep}", bufs=4, space="PSUM") as ffps,
        tc.tile_pool(name=f"ff2ps_{rep}", bufs=1, space="PSUM") as ff2ps,
    ):
        for g in range(8):
            w1_sb = c.w1s.tile([128, 4, KT, 128], F16, tag="w1", name=f"w1_{rep}_{g}")
            nc.sync.dma_start(w1_sb[:], c.W1g[g])
            for mi in range(4):
                m = g * 4 + mi
                ps_f = ffps.tile([128, SL], F32, tag="ff_ps", name=f"ff_ps_{rep}_{m}")
                for k in range(KT):
                    nc.tensor.matmul(
                        ps_f[:], w1_sb[:, mi, k, :], s["xT"][:, k, :],
                        start=(k == 0), stop=(k == KT - 1),
                    )
                nc.scalar.activation(ffh_sb[:, m, :], ps_f[:], AF.Relu,
                                     bias=c.b1_sb[:, m:m + 1], scale=1.0)

        # swapped FFN2: psum [s, d-chunk] = ffh_tile.T @ W2T_tile
        ps_g = [ff2ps.tile([128, SCH], F32, tag=f"ff2_ps{i}", name=f"ff2_ps_{rep}_{i}")
                for i in range(4)]
        for g in range(8):
            w2_sb = c.w2s.tile([128, 4, D], F16, tag="w2", name=f"w2_{rep}_{g}")
            nc.sync.dma_start(w2_sb[:], c.W2g[g])
            for ki in range(4):
                k = g * 4 + ki
                for st in range(2):
                    for dc in range(2):
                        nc.tensor.matmul(
                            ps_g[st * 2 + dc][:],
                            ffh_sb[:, k, st * 128:(st + 1) * 128],
                            w2_sb[:, ki, dc * SCH:(dc + 1) * SCH],
                            start=(k == 0), stop=(k == FFT - 1),
                        )
        for st in range(2):
            x2_sb = c.lnsb.tile([128, D], F32, tag="x2", name=f"x2_{rep}_{st}")
            for dc in range(2):
                nc.vector.tensor_add(
                    out=x2_sb[:, dc * SCH:(dc + 1) * SCH],
                    in0=ps_g[st * 2 + dc][:],
                    in1=s["z"][:, st, dc * SCH:(dc + 1) * SCH],
                )
            nc.vector.tensor_add(out=x2_sb[:], in0=x2_sb[:], in1=c.b2f_sb[:])
            _ln(nc, c.lnsb, x2_sb, x2_sb, c.a2_sb, c.b2n_sb, f"{rep}_2_{st}")
            nc.sync.dma_start(c.outs[rep].ap()[st], x2_sb[:])


_NC_CACHE = {}


def _get_nc(reps: int = 1):
    if reps not in _NC_CACHE:
        _NC_CACHE[reps] = build(reps)
    return _NC_CACHE[reps]


def prep_inputs(Q, K, V, wq, wk, wv, Wo, W1, b1, W2, b2, a_2, b_2):
    """Host-side sharding/layout prep. Returns per-core input maps."""
    f32 = np.float32
    bf = BF16NP
    Q = np.asarray(Q, f32); K = np.asarray(K, f32); V = np.asarray(V, f32)
    Qt = np.ascontiguousarray(Q.reshape(KT, 128, S).astype(F16NP))
    Kt = np.ascontiguousarray(K.reshape(KT, 128, S).astype(F16NP))
    Vt = np.ascontiguousarray(V.reshape(KT, 128, S).astype(bf))
    Wot = np.ascontiguousarray(
        np.asarray(Wo, f32).reshape(KT, 128, D).transpose(1, 0, 2).astype(F16NP))
    # FFN1 consumes the pre-affine normalized activations: fold LN1's affine
    # into the weights/bias: W1' = W1 * a2 (per input column), b1' = b1 + W1 @ b2
    W1p = np.asarray(W1, f32) * np.asarray(a_2, f32)[None, :]
    b1 = np.asarray(b1, f32) + np.asarray(W1, f32) @ np.asarray(b_2, f32)
    # W1g: [8, 128(ki), 4(m-sub), 8(kt), 128(m)]
    W1g = np.ascontiguousarray(
        W1p.reshape(8, 4, 128, KT, 128)  # [g, msub, m, kt, ki]
        .transpose(0, 4, 1, 3, 2).astype(F16NP))             # -> [g, ki, msub, kt, m]
    # W2g: [8, 128(ki over f), 4(kt-sub over f), 1024(d)]
    W2g = np.ascontiguousarray(
        np.asarray(W2, f32).T.reshape(8, 4, 128, D).transpose(0, 2, 1, 3).astype(F16NP))
    b1t = np.ascontiguousarray(np.asarray(b1, f32).reshape(FFT, 128).T)
    b2vr = np.asarray(b2, f32).reshape(1, D)
    a2vr = np.asarray(a_2, f32).reshape(1, D)
    b2nvr = np.asarray(b_2, f32).reshape(1, D)

    wq = np.asarray(wq, f32); wk = np.asarray(wk, f32); wv = np.asarray(wv, f32)
    in_maps = []
    for c in range(NCORES):
        def _wT(w, dt=bf):
            wc = w[c * HPC:(c + 1) * HPC].reshape(128, D)  # [m, k]
            return np.ascontiguousarray(
                wc.reshape(128, KT, 128).transpose(2, 1, 0).astype(dt))
        VsT = np.ascontiguousarray(V[:, c * SL:(c + 1) * SL].T.reshape(2, 128, D))
        in_maps.append({
            "Qt": Qt, "Kt": Kt, "Vt": Vt,
            "wqT": _wT(wq, F16NP), "wkT": _wT(wk, F16NP), "wvT": _wT(wv),
            "Wot": Wot, "W1g": W1g, "W2g": W2g,
            "b1t": b1t, "b2v": b2vr, "a2v": a2vr, "b2nv": b2nvr,
            "VsT": VsT,
        })
    return in_maps


def run(in_maps, reps: int = 1):
    nc = _get_nc(reps)
    return run_bass_kernel_spmd(nc, in_maps, list(range(NCORES)))


def assemble(results, rep=0):
    """[2,128,1024] per core -> full [1024, 2048] output."""
    z2 = np.concatenate(
        [results[c][f"out{rep}"].reshape(2 * 128, D) for c in range(NCORES)], axis=0)
    return np.ascontiguousarray(z2.T)


def kernel(Q, K, V, wq, wk, wv, Wo, W1, b1, W2, b2, a_2, b_2):
    in_maps = prep_inputs(Q, K, V, wq, wk, wv, Wo, W1, b1, W2, b2, a_2, b_2)
    res = run(in_maps, reps=1).results
    return assemble(res)
